# revision 16
# baseline (speedup 1.0000x reference)
"""Converged Toeplitz inhibition kernel for TRN2 (8 NeuronCores, SPMD).

out[n, c, h, w] = sum_k act[n, k, h, w] * Winv[k, c]
where Winv = inv(I - circulant(pad_roll(inhibition_filter, C)))  [C x C]

Strategy: Winv = I + E with ||E|| small (max entry 0.064, max column norm
0.18), because the inhibition coupling is weak.  Split the product:

    out = act + act @ E          (identity part exact, correction small)

The identity part is added on the host in fp32 (exact).  The device
computes the full dense correction in fp8:

  - act is cast to fp8 e4m3 on the host (error feeds only the correction,
    scaled by ||E|| ~ 0.18, so it is harmless)
  - E is scaled by 2^11 so all its entries sit in e4m3's normal range
    (max 128 < 240; unscaled, half its entries would be subnormal)
  - matmuls run in DoubleRow perf mode: fp8 pairs double the contraction
    depth per partition (K=256 in ONE 512-col matmul) and double-pump the
    PE; measured issue rate ~256 ns per [K256 x M128 x N512] matmul
  - PSUM is evacuated with a fused scale (x 2^3 / 2^11) and cast to e3m4
    (4 mantissa bits; corr*8 max ~8.8 < 15.5 so no saturation)
  - host: out = act_f32 + corr_e3m4 * (1/8)

Measured rel err 8.2e-3 (gate 2e-2); wire traffic 4.19 MB in + 4.19 MB
out per core (1 byte/element each way).

Schedule (from trace analysis of the first fp8 cut, 46.6 us):
  - fixed framework preamble ~7.2 us (engine barriers + library loads)
    and teardown ~4 us; nothing issued before ~7.2 us ever runs
  - the steady-state limiter is PSUM evacuation: ACT/DVE read fp32 PSUM
    at ~1.3-1.5 us per [128, 1024] tile (fp32 operand disables all DVE 2x
    modes), so evac is split THREE ways: ScalarE (1.335 us/tile), VectorE
    (1.46 us/tile) and GpSimd (~2.1 us/tile, 0.42 sw efficiency), weighted
    12/12/8 -> ~17.4 us of evac vs 22.4 us for the two-engine split
  - weights load FIRST on the sync HWDGE ring (64 KB, lands ~7.4 us),
    then all 16 input chunks stream on the same ring back-to-back (whole
    fp8 input = 32 KB/partition, fits SBUF, so no reuse stalls); a DMA
    transfer does not block its issuing engine (verified in trace), the
    ring just serializes its own transfers at ~350 GB/s
  - out-DMAs alternate the scalar/vector HWDGE rings (trigger cost on the
    engine is tiny; GpSimd SWDGE would burn Pool-engine descriptor-gen
    time that evac now needs); the last batch drains over sync+scalar+
    vector rings
  - a few warmup matmuls over the weight tile bridge the 7.4 -> 8.8 us
    window before the first act chunk lands (HAM throttle ramp)
"""

import numpy as np
import ml_dtypes

import concourse.bass as bass
import concourse.bacc as bacc
import concourse.mybir as mybir
import concourse.tile as tile
from concourse.bass_utils import run_bass_kernel_spmd

N, C, H, W = 32, 256, 64, 64
HW = H * W  # 4096
NCORES = 8
NB = N // NCORES  # batches per core
P = 128  # partitions
FD = 512  # matmul free dim (one fp32 PSUM bank)
CH = 2048  # chunk width (columns)

IN_DT = mybir.dt.float8e4  # e4m3: act + weights (DoubleRow needs e4/e5)
OUT_DT = mybir.dt.float8e3  # e3m4: correction output
SW = 2048.0  # weight scale (E*SW max ~130, all entries normal-range)
SO = 8.0  # output scale  (corr*SO max ~8.8 < 15.5)

NP_IN = ml_dtypes.float8_e4m3
NP_OUT = ml_dtypes.float8_e3m4


def _build_w(inhibition_filter: np.ndarray) -> np.ndarray:
    """Replicates reference._pad_roll + _circulant + inv(I - tpl) in numpy."""
    filt = np.asarray(inhibition_filter, dtype=np.float32)
    scope = filt.shape[0]
    pad_left = (C - scope) // 2
    padded = np.zeros(C, np.float32)
    padded[pad_left : pad_left + scope] = filt
    kernel = np.roll(padded, C // 2 + 1)
    idx = (np.arange(C)[None, :] - np.arange(C)[:, None]) % C
    tpl = kernel[idx].astype(np.float64)
    w = np.linalg.inv(np.eye(C, dtype=np.float64) - tpl)
    return np.ascontiguousarray(w.astype(np.float32))


# GPSIMD cannot read PSUM (BIR verifier), so evacuation is strictly
# ScalarE+VectorE.  ACT is ~9% faster per tile, so it takes 17 of the 32
# psum halves and DVE 15.
CFG = {
    # Warmups bridge weights-ready (~9.4 us) to the first chunk's DMA
    # *completion* (~12.5 us) and keep the PE clock ramping; 6 of them
    # end just before the data lands.
    "nwarm": 6,
    # 17 scalar / 15 vector halves; one extra scalar half early (while
    # the PE is still ramping and scalar would idle anyway), one at the
    # very end, so both engines finish together.
    "evac_pat": "ssv" + "sv" * 14 + "s",
    "out_pat": "g",  # bulk out-DMAs: gpsimd SWDGE (Pool engine is idle)
    "drain_pat": "gy",  # last drain DMA rides sync (no SWDGE desc latency)
}

_ENG = {"s": "scalar", "v": "vector", "g": "gpsimd", "y": "sync"}


def _body(tc: tile.TileContext, out, act, w, cfg=None):
    cfg = dict(CFG, **(cfg or {}))
    nc = tc.nc
    NCH = HW // CH  # chunks per batch
    DR = mybir.MatmulPerfMode.DoubleRow
    evac_engines = [getattr(nc, _ENG[ch]) for ch in cfg["evac_pat"]]
    out_rings = [getattr(nc, _ENG[ch]) for ch in cfg["out_pat"]]
    drain_rings = [getattr(nc, _ENG[ch]) for ch in cfg["drain_pat"]]

    def evac(eng, dst, src, scale):
        # fused fp32 -> e3m4 cast with scale; ACT uses activation-Copy,
        # DVE/Pool use tensor_scalar multiply
        if eng is nc.scalar:
            eng.mul(dst, src, scale)
        else:
            eng.tensor_scalar_mul(dst, src, scale)

    with (
        tc.tile_pool(name="wpool", bufs=1) as wpool,
        tc.tile_pool(name="apool", bufs=1) as apool,
        tc.tile_pool(name="opool", bufs=2) as opool,
        tc.tile_pool(name="psum", bufs=2, space="PSUM") as pspool,
    ):
        # Weights [128, 2, 256]: wtile[p, i, m] = E[i*128+p, m] * SW.
        # First on the sync ring so it lands before any act chunk.
        # DMA-completion semaphores are delivered ~1.15 us/DMA behind the
        # transfers; 16 x 256 KB chunks measured best (fewer/larger DMAs
        # delay the first completion, more/smaller ones delay the rest).
        wtile = wpool.tile([P, 2, C], IN_DT, tag="w", name="wtile")
        nc.sync.dma_start(out=wtile[:], in_=w[:, :, :])

        # All input chunks up front: the whole fp8 input (32 KB/partition)
        # fits SBUF, so every chunk gets its own buffer and the sync ring
        # streams with no reuse stalls.
        a = {}
        for n in range(NB):
            for c in range(NCH):
                a[n, c] = apool.tile(
                    [P, 2, CH], IN_DT, tag=f"a{n}{c}", name=f"a{n}{c}"
                )
                # h0 on the sync ring, h1 on the scalar ring: the two
                # completion pipelines run in parallel, so a chunk's pair
                # of transfers completes ~2x sooner.  The ACT engine only
                # pays the trigger issue (early, while it is idle).
                nc.sync.dma_start(
                    out=a[n, c][:, 0, :],
                    in_=act[n, 0, :, c * CH : (c + 1) * CH],
                )
                nc.scalar.dma_start(
                    out=a[n, c][:, 1, :],
                    in_=act[n, 1, :, c * CH : (c + 1) * CH],
                )

        # PE warmup over the weight tile itself (no uninitialized reads).
        for i in range(cfg["nwarm"]):
            pw = pspool.tile(
                [P, 2 * FD], mybir.dt.float32, tag=f"ps{'AB'[i % 2]}", name="pw"
            )
            nc.tensor.matmul(
                pw[:, 0:C],
                lhsT=wtile[:, :, 0:P],
                rhs=wtile[:, :, :],
                start=True,
                stop=True,
                perf_mode=DR,
            )

        ecnt = 0  # evac-engine cursor
        ocnt = 0  # out-ring cursor
        for n in range(NB):
            last = n == NB - 1
            for c in range(NCH):
                for m in range(2):
                    o = opool.tile(
                        [P, CH], OUT_DT, tag=f"o{m}{c}", name=f"o{n}{m}{c}",
                        bufs=2,
                    )
                    # Two 1-bank-deep [128, 1024] PSUM tiles per (c, m);
                    # each takes two single-shot DoubleRow matmuls (full
                    # K=256 contraction per instruction).
                    ps = [
                        pspool.tile(
                            [P, 2 * FD], mybir.dt.float32, tag=f"ps{'AB'[h]}",
                            name="ps",
                        )
                        for h in range(2)
                    ]
                    for h in range(2):
                        for jj in range(2):
                            j = h * 2 + jj
                            nc.tensor.matmul(
                                ps[h][:, jj * FD : (jj + 1) * FD],
                                lhsT=wtile[:, :, m * P : (m + 1) * P],
                                rhs=a[n, c][:, :, j * FD : (j + 1) * FD],
                                start=True,
                                stop=True,
                                perf_mode=DR,
                            )
                    # Evacuate fp32 -> e3m4 with fused *SO/SW; the two
                    # halves go to the next two engines in the weighted
                    # rotation (scalar/vector/gpsimd 12/12/8).
                    for h in range(2):
                        evac(
                            evac_engines[ecnt % len(evac_engines)],
                            o[:, h * 2 * FD : (h + 1) * 2 * FD],
                            ps[h][:],
                            SO / SW,
                        )
                        ecnt += 1
                    rings = drain_rings if last else out_rings
                    ring = rings[ocnt % len(rings)]
                    ocnt += 1
                    ring.dma_start(
                        out=out[n, m, :, c * CH : (c + 1) * CH],
                        in_=o[:],
                    )


_NC_CACHE = {}


def _get_nc(cfg=None):
    key = tuple(sorted(dict(CFG, **(cfg or {})).items()))
    if key not in _NC_CACHE:
        nc = bacc.Bacc(
            "TRN2", debug=False, enable_asserts=False, enable_partition_id=False
        )
        act = nc.dram_tensor("act", [NB, 2, P, HW], IN_DT, kind="ExternalInput").ap()
        w = nc.dram_tensor("w", [P, 2, C], IN_DT, kind="ExternalInput").ap()
        out = nc.dram_tensor("out", [NB, 2, P, HW], OUT_DT, kind="ExternalOutput").ap()
        with tile.TileContext(nc) as tc:
            _body(tc, out, act, w, cfg)
        nc.compile()
        _NC_CACHE[key] = nc
    return _NC_CACHE[key]


def _run(activations: np.ndarray, w: np.ndarray, trace: bool = False, cfg=None):
    act32 = np.ascontiguousarray(activations, dtype=np.float32)
    acts8 = act32.reshape(NCORES, NB, 2, P, HW).astype(NP_IN)
    # E = Winv - I, scaled into e4m3 normal range and packed [128, 2, 256]:
    # wp[p, i, m] = E[i*128+p, m] * SW.
    E = (w.astype(np.float64) - np.eye(C)) * SW
    wp = np.ascontiguousarray(
        E.astype(np.float32).reshape(2, P, C).transpose(1, 0, 2).astype(NP_IN)
    )
    in_maps = [{"act": acts8[i], "w": wp} for i in range(NCORES)]
    nc = _get_nc(cfg)
    res = run_bass_kernel_spmd(nc, in_maps, list(range(NCORES)), trace=trace)
    corr = np.stack([res.results[i]["out"] for i in range(NCORES)], axis=0)
    out = act32 + corr.astype(np.float32).reshape(N, C, H, W) * np.float32(1.0 / SO)
    return out, res


def kernel(activations: np.ndarray, inhibition_filter: np.ndarray) -> np.ndarray:
    w = _build_w(inhibition_filter)
    out, _ = _run(activations, w, trace=False)
    return out


# revision 18
# speedup vs baseline: 1.0922x; 1.0922x over previous
"""Converged Toeplitz inhibition kernel for TRN2 (8 NeuronCores, SPMD).

out[n, c, h, w] = sum_k act[n, k, h, w] * Winv[k, c]
where Winv = inv(I - circulant(pad_roll(inhibition_filter, C)))  [C x C]

Strategy: Winv = I + E with ||E|| small (max entry 0.064, max column norm
0.18), because the inhibition coupling is weak.  Split the product:

    out = act + act @ E          (identity part exact, correction small)

The identity part is added on the host in fp32 (exact).  The device
computes the full dense correction in fp8:

  - act is cast to fp8 e4m3 on the host (error feeds only the correction,
    scaled by ||E|| ~ 0.18, so it is harmless)
  - E is scaled by 2^11 so all its entries sit in e4m3's normal range
    (max 128 < 240; unscaled, half its entries would be subnormal)
  - matmuls run in DoubleRow perf mode: fp8 pairs double the contraction
    depth per partition (K=256 in ONE 512-col matmul) and double-pump the
    PE; measured issue rate ~256 ns per [K256 x M128 x N512] matmul
  - PSUM is evacuated with a fused scale (x 2^3 / 2^11) and cast to e3m4
    (4 mantissa bits; corr*8 max ~8.8 < 15.5 so no saturation)
  - host: out = act_f32 + corr_e3m4 * (1/8)

Measured rel err 8.2e-3 (gate 2e-2); wire traffic 4.19 MB in + 4.19 MB
out per core (1 byte/element each way).

Schedule (from trace analysis of the first fp8 cut, 46.6 us):
  - fixed framework preamble ~7.2 us (engine barriers + library loads)
    and teardown ~4 us; nothing issued before ~7.2 us ever runs
  - the steady-state limiter is PSUM evacuation: ACT/DVE read fp32 PSUM
    at ~1.3-1.5 us per [128, 1024] tile (fp32 operand disables all DVE 2x
    modes), so evac is split THREE ways: ScalarE (1.335 us/tile), VectorE
    (1.46 us/tile) and GpSimd (~2.1 us/tile, 0.42 sw efficiency), weighted
    12/12/8 -> ~17.4 us of evac vs 22.4 us for the two-engine split
  - weights load FIRST on the sync HWDGE ring (64 KB, lands ~7.4 us),
    then all 16 input chunks stream on the same ring back-to-back (whole
    fp8 input = 32 KB/partition, fits SBUF, so no reuse stalls); a DMA
    transfer does not block its issuing engine (verified in trace), the
    ring just serializes its own transfers at ~350 GB/s
  - out-DMAs alternate the scalar/vector HWDGE rings (trigger cost on the
    engine is tiny; GpSimd SWDGE would burn Pool-engine descriptor-gen
    time that evac now needs); the last batch drains over sync+scalar+
    vector rings
  - a few warmup matmuls over the weight tile bridge the 7.4 -> 8.8 us
    window before the first act chunk lands (HAM throttle ramp)
"""

import numpy as np
import ml_dtypes

import concourse.bass as bass
import concourse.bacc as bacc
import concourse.mybir as mybir
import concourse.tile as tile
from concourse.bass_utils import run_bass_kernel_spmd

N, C, H, W = 32, 256, 64, 64
HW = H * W  # 4096
NCORES = 8
NB = N // NCORES  # batches per core
P = 128  # partitions
FD = 512  # matmul free dim (one fp32 PSUM bank)
CH = 2048  # chunk width (columns)

IN_DT = mybir.dt.float8e4  # e4m3: act + weights (DoubleRow needs e4/e5)
OUT_DT = mybir.dt.float8e3  # e3m4: correction output
SW = 2048.0  # weight scale (E*SW max ~130, all entries normal-range)
SO = 8.0  # output scale  (corr*SO max ~8.8 < 15.5)

NP_IN = ml_dtypes.float8_e4m3
NP_OUT = ml_dtypes.float8_e3m4


def _build_w(inhibition_filter: np.ndarray) -> np.ndarray:
    """Replicates reference._pad_roll + _circulant + inv(I - tpl) in numpy."""
    filt = np.asarray(inhibition_filter, dtype=np.float32)
    scope = filt.shape[0]
    pad_left = (C - scope) // 2
    padded = np.zeros(C, np.float32)
    padded[pad_left : pad_left + scope] = filt
    kernel = np.roll(padded, C // 2 + 1)
    idx = (np.arange(C)[None, :] - np.arange(C)[:, None]) % C
    tpl = kernel[idx].astype(np.float64)
    w = np.linalg.inv(np.eye(C, dtype=np.float64) - tpl)
    return np.ascontiguousarray(w.astype(np.float32))


# GPSIMD cannot read PSUM (BIR verifier), so evacuation is strictly
# ScalarE+VectorE.  ACT is ~9% faster per tile, so it takes 17 of the 32
# psum halves and DVE 15.
CFG = {
    # Warmups bridge weights-ready (~9.4 us) to the first chunk's DMA
    # *completion* (~12.5 us) and keep the PE clock ramping; 6 of them
    # end just before the data lands.
    "nwarm": 2,
    # 17 scalar / 15 vector halves; one extra scalar half early (while
    # the PE is still ramping and scalar would idle anyway), one at the
    # very end, so both engines finish together.
    "evac_pat": "ssv" + "sv" * 14 + "s",
    "out_pat": "g",  # bulk out-DMAs: gpsimd SWDGE (Pool engine is idle)
    "drain_pat": "gy",  # last drain DMA rides sync (no SWDGE desc latency)
}

_ENG = {"s": "scalar", "v": "vector", "g": "gpsimd", "y": "sync"}


def _body(tc: tile.TileContext, out, act, w, cfg=None):
    cfg = dict(CFG, **(cfg or {}))
    nc = tc.nc
    NCH = HW // CH  # chunks per batch
    DR = mybir.MatmulPerfMode.DoubleRow
    evac_engines = [getattr(nc, _ENG[ch]) for ch in cfg["evac_pat"]]
    out_rings = [getattr(nc, _ENG[ch]) for ch in cfg["out_pat"]]
    drain_rings = [getattr(nc, _ENG[ch]) for ch in cfg["drain_pat"]]

    def evac(eng, dst, src, scale):
        # fused fp32 -> e3m4 cast with scale; ACT uses activation-Copy,
        # DVE/Pool use tensor_scalar multiply
        if eng is nc.scalar:
            eng.mul(dst, src, scale)
        else:
            eng.tensor_scalar_mul(dst, src, scale)

    with (
        tc.tile_pool(name="wpool", bufs=1) as wpool,
        tc.tile_pool(name="apool", bufs=1) as apool,
        tc.tile_pool(name="opool", bufs=2) as opool,
        tc.tile_pool(name="psum", bufs=2, space="PSUM") as pspool,
    ):
        # Weights [128, 2, 256]: wtile[p, i, m] = E[i*128+p, m] * SW.
        # First on the sync ring so it lands before any act chunk.
        # DMA-completion semaphores are delivered ~1.15 us/DMA behind the
        # transfers; 16 x 256 KB chunks measured best (fewer/larger DMAs
        # delay the first completion, more/smaller ones delay the rest).
        wtile = wpool.tile([P, 2, C], IN_DT, tag="w", name="wtile")
        nc.sync.dma_start(out=wtile[:], in_=w[:, :, :])

        # All input chunks up front: the whole fp8 input (32 KB/partition)
        # fits SBUF, so every chunk gets its own buffer and the sync ring
        # streams with no reuse stalls.
        a = {}
        for n in range(NB):
            for c in range(NCH):
                a[n, c] = apool.tile(
                    [P, 2, CH], IN_DT, tag=f"a{n}{c}", name=f"a{n}{c}"
                )
                # One 3D DMA per chunk (the DRAM act layout is
                # [NB, 128, 2, HW] so both k-halves of a chunk transfer
                # as one [128, 2, CH] pattern): 9 input completions
                # instead of 17, so the first chunk is consumable ~2 us
                # sooner and the queue drains in half the time.
                nc.sync.dma_start(
                    out=a[n, c][:, :, :],
                    in_=act[n, :, :, c * CH : (c + 1) * CH],
                )

        # PE warmup over the weight tile itself (no uninitialized reads).
        for i in range(cfg["nwarm"]):
            pw = pspool.tile(
                [P, 2 * FD], mybir.dt.float32, tag=f"ps{'AB'[i % 2]}", name="pw"
            )
            nc.tensor.matmul(
                pw[:, 0:C],
                lhsT=wtile[:, :, 0:P],
                rhs=wtile[:, :, :],
                start=True,
                stop=True,
                perf_mode=DR,
            )

        ecnt = 0  # evac-engine cursor
        ocnt = 0  # out-ring cursor
        for n in range(NB):
            last = n == NB - 1
            for c in range(NCH):
                for m in range(2):
                    o = opool.tile(
                        [P, CH], OUT_DT, tag=f"o{m}{c}", name=f"o{n}{m}{c}",
                        bufs=2,
                    )
                    # Two 1-bank-deep [128, 1024] PSUM tiles per (c, m);
                    # each takes two single-shot DoubleRow matmuls (full
                    # K=256 contraction per instruction).
                    ps = [
                        pspool.tile(
                            [P, 2 * FD], mybir.dt.float32, tag=f"ps{'AB'[h]}",
                            name="ps",
                        )
                        for h in range(2)
                    ]
                    for h in range(2):
                        for jj in range(2):
                            j = h * 2 + jj
                            nc.tensor.matmul(
                                ps[h][:, jj * FD : (jj + 1) * FD],
                                lhsT=wtile[:, :, m * P : (m + 1) * P],
                                rhs=a[n, c][:, :, j * FD : (j + 1) * FD],
                                start=True,
                                stop=True,
                                perf_mode=DR,
                            )
                    # Evacuate fp32 -> e3m4 with fused *SO/SW; the two
                    # halves go to the next two engines in the weighted
                    # rotation (scalar/vector/gpsimd 12/12/8).
                    for h in range(2):
                        evac(
                            evac_engines[ecnt % len(evac_engines)],
                            o[:, h * 2 * FD : (h + 1) * 2 * FD],
                            ps[h][:],
                            SO / SW,
                        )
                        ecnt += 1
                    rings = drain_rings if last else out_rings
                    ring = rings[ocnt % len(rings)]
                    ocnt += 1
                    ring.dma_start(
                        out=out[n, m, :, c * CH : (c + 1) * CH],
                        in_=o[:],
                    )


_NC_CACHE = {}


def _get_nc(cfg=None):
    key = tuple(sorted(dict(CFG, **(cfg or {})).items()))
    if key not in _NC_CACHE:
        nc = bacc.Bacc(
            "TRN2", debug=False, enable_asserts=False, enable_partition_id=False
        )
        act = nc.dram_tensor("act", [NB, P, 2, HW], IN_DT, kind="ExternalInput").ap()
        w = nc.dram_tensor("w", [P, 2, C], IN_DT, kind="ExternalInput").ap()
        out = nc.dram_tensor("out", [NB, 2, P, HW], OUT_DT, kind="ExternalOutput").ap()
        with tile.TileContext(nc) as tc:
            _body(tc, out, act, w, cfg)
        nc.compile()
        _NC_CACHE[key] = nc
    return _NC_CACHE[key]


def _run(activations: np.ndarray, w: np.ndarray, trace: bool = False, cfg=None):
    act32 = np.ascontiguousarray(activations, dtype=np.float32)
    acts8 = np.ascontiguousarray(
        act32.reshape(NCORES, NB, 2, P, HW).transpose(0, 1, 3, 2, 4)
    ).astype(NP_IN)
    # E = Winv - I, scaled into e4m3 normal range and packed [128, 2, 256]:
    # wp[p, i, m] = E[i*128+p, m] * SW.
    E = (w.astype(np.float64) - np.eye(C)) * SW
    wp = np.ascontiguousarray(
        E.astype(np.float32).reshape(2, P, C).transpose(1, 0, 2).astype(NP_IN)
    )
    in_maps = [{"act": acts8[i], "w": wp} for i in range(NCORES)]
    nc = _get_nc(cfg)
    res = run_bass_kernel_spmd(nc, in_maps, list(range(NCORES)), trace=trace)
    corr = np.stack([res.results[i]["out"] for i in range(NCORES)], axis=0)
    out = act32 + corr.astype(np.float32).reshape(N, C, H, W) * np.float32(1.0 / SO)
    return out, res


def kernel(activations: np.ndarray, inhibition_filter: np.ndarray) -> np.ndarray:
    w = _build_w(inhibition_filter)
    out, _ = _run(activations, w, trace=False)
    return out


# revision 20
# speedup vs baseline: 1.1298x; 1.0345x over previous
"""Converged Toeplitz inhibition kernel for TRN2 (8 NeuronCores, SPMD).

out[n, c, h, w] = sum_k act[n, k, h, w] * Winv[k, c]
where Winv = inv(I - circulant(pad_roll(inhibition_filter, C)))  [C x C]

Strategy: Winv = I + E with ||E|| small (max entry 0.064, max column norm
0.18), because the inhibition coupling is weak.  Split the product:

    out = act + act @ E          (identity part exact, correction small)

The identity part is added on the host in fp32 (exact).  The device
computes the full dense correction in fp8:

  - act is cast to fp8 e4m3 on the host (error feeds only the correction,
    scaled by ||E|| ~ 0.18, so it is harmless)
  - E is scaled by 2^11 so all its entries sit in e4m3's normal range
    (max 128 < 240; unscaled, half its entries would be subnormal)
  - matmuls run in DoubleRow perf mode: fp8 pairs double the contraction
    depth per partition (K=256 in ONE 512-col matmul) and double-pump the
    PE; measured issue rate ~256 ns per [K256 x M128 x N512] matmul
  - PSUM is evacuated with a fused scale (x 2^3 / 2^11) and cast to e3m4
    (4 mantissa bits; corr*8 max ~8.8 < 15.5 so no saturation)
  - host: out = act_f32 + corr_e3m4 * (1/8)

Measured rel err 8.2e-3 (gate 2e-2); wire traffic 4.19 MB in + 4.19 MB
out per core (1 byte/element each way).

Schedule (from trace analysis of the first fp8 cut, 46.6 us):
  - fixed framework preamble ~7.2 us (engine barriers + library loads)
    and teardown ~4 us; nothing issued before ~7.2 us ever runs
  - the steady-state limiter is PSUM evacuation: ACT/DVE read fp32 PSUM
    at ~1.3-1.5 us per [128, 1024] tile (fp32 operand disables all DVE 2x
    modes), so evac is split THREE ways: ScalarE (1.335 us/tile), VectorE
    (1.46 us/tile) and GpSimd (~2.1 us/tile, 0.42 sw efficiency), weighted
    12/12/8 -> ~17.4 us of evac vs 22.4 us for the two-engine split
  - weights load FIRST on the sync HWDGE ring (64 KB, lands ~7.4 us),
    then all 16 input chunks stream on the same ring back-to-back (whole
    fp8 input = 32 KB/partition, fits SBUF, so no reuse stalls); a DMA
    transfer does not block its issuing engine (verified in trace), the
    ring just serializes its own transfers at ~350 GB/s
  - out-DMAs alternate the scalar/vector HWDGE rings (trigger cost on the
    engine is tiny; GpSimd SWDGE would burn Pool-engine descriptor-gen
    time that evac now needs); the last batch drains over sync+scalar+
    vector rings
  - a few warmup matmuls over the weight tile bridge the 7.4 -> 8.8 us
    window before the first act chunk lands (HAM throttle ramp)
"""

import numpy as np
import ml_dtypes

import concourse.bass as bass
import concourse.bacc as bacc
import concourse.mybir as mybir
import concourse.tile as tile
from concourse.bass_utils import run_bass_kernel_spmd

N, C, H, W = 32, 256, 64, 64
HW = H * W  # 4096
NCORES = 8
NB = N // NCORES  # batches per core
P = 128  # partitions
FD = 512  # matmul free dim (one fp32 PSUM bank)
CH = 2048  # chunk width (columns)

IN_DT = mybir.dt.float8e4  # e4m3: act + weights (DoubleRow needs e4/e5)
OUT_DT = mybir.dt.float8e3  # e3m4: correction output
SW = 2048.0  # weight scale (E*SW max ~130, all entries normal-range)
SO = 8.0  # output scale  (corr*SO max ~8.8 < 15.5)

NP_IN = ml_dtypes.float8_e4m3
NP_OUT = ml_dtypes.float8_e3m4


def _build_w(inhibition_filter: np.ndarray) -> np.ndarray:
    """Replicates reference._pad_roll + _circulant + inv(I - tpl) in numpy."""
    filt = np.asarray(inhibition_filter, dtype=np.float32)
    scope = filt.shape[0]
    pad_left = (C - scope) // 2
    padded = np.zeros(C, np.float32)
    padded[pad_left : pad_left + scope] = filt
    kernel = np.roll(padded, C // 2 + 1)
    idx = (np.arange(C)[None, :] - np.arange(C)[:, None]) % C
    tpl = kernel[idx].astype(np.float64)
    w = np.linalg.inv(np.eye(C, dtype=np.float64) - tpl)
    return np.ascontiguousarray(w.astype(np.float32))


# GPSIMD cannot read PSUM (BIR verifier), so evacuation is strictly
# ScalarE+VectorE.  ACT is ~9% faster per tile, so it takes 17 of the 32
# psum halves and DVE 15.
CFG = {
    # Warmups bridge weights-ready (~9.4 us) to the first chunk's DMA
    # *completion* (~12.5 us) and keep the PE clock ramping; 6 of them
    # end just before the data lands.
    "nwarm": 6,
    # 17 scalar / 15 vector halves; one extra scalar half early (while
    # the PE is still ramping and scalar would idle anyway), one at the
    # very end, so both engines finish together.
    "evac_pat": "ssv" + "sv" * 14 + "s",
    "out_pat": "g",  # bulk out-DMAs: gpsimd SWDGE (Pool engine is idle)
    "drain_pat": "gy",  # last drain DMA rides sync (no SWDGE desc latency)
}

_ENG = {"s": "scalar", "v": "vector", "g": "gpsimd", "y": "sync"}


def _body(tc: tile.TileContext, out, act, w, cfg=None):
    cfg = dict(CFG, **(cfg or {}))
    nc = tc.nc
    NCH = HW // CH  # chunks per batch
    DR = mybir.MatmulPerfMode.DoubleRow
    evac_engines = [getattr(nc, _ENG[ch]) for ch in cfg["evac_pat"]]
    out_rings = [getattr(nc, _ENG[ch]) for ch in cfg["out_pat"]]
    drain_rings = [getattr(nc, _ENG[ch]) for ch in cfg["drain_pat"]]

    def evac(eng, dst, src, scale):
        # fused fp32 -> e3m4 cast with scale; ACT uses activation-Copy,
        # DVE/Pool use tensor_scalar multiply
        if eng is nc.scalar:
            eng.mul(dst, src, scale)
        else:
            eng.tensor_scalar_mul(dst, src, scale)

    with (
        tc.tile_pool(name="wpool", bufs=1) as wpool,
        tc.tile_pool(name="apool", bufs=1) as apool,
        tc.tile_pool(name="opool", bufs=2) as opool,
        tc.tile_pool(name="psum", bufs=2, space="PSUM") as pspool,
    ):
        # Weights [128, 2, 256]: wtile[p, i, m] = E[i*128+p, m] * SW.
        # First on the sync ring so it lands before any act chunk.
        # DMA-completion semaphores are delivered ~1.15 us/DMA behind the
        # transfers; 16 x 256 KB chunks measured best (fewer/larger DMAs
        # delay the first completion, more/smaller ones delay the rest).
        wtile = wpool.tile([P, 2, C], IN_DT, tag="w", name="wtile")
        nc.sync.dma_start(out=wtile[:], in_=w[:, :, :])

        # All input chunks up front: the whole fp8 input (32 KB/partition)
        # fits SBUF, so every chunk gets its own buffer and the sync ring
        # streams with no reuse stalls.
        a = {}
        for n in range(NB):
            for c in range(NCH):
                a[n, c] = apool.tile(
                    [P, 2, CH], IN_DT, tag=f"a{n}{c}", name=f"a{n}{c}"
                )
                for h in range(2):
                    nc.sync.dma_start(
                        out=a[n, c][:, h, :],
                        in_=act[n, h, :, c * CH : (c + 1) * CH],
                    )

        # PE warmup over the weight tile itself (no uninitialized reads).
        for i in range(cfg["nwarm"]):
            pw = pspool.tile(
                [P, 2 * FD], mybir.dt.float32, tag=f"ps{'AB'[i % 2]}", name="pw"
            )
            nc.tensor.matmul(
                pw[:, 0:C],
                lhsT=wtile[:, :, 0:P],
                rhs=wtile[:, :, :],
                start=True,
                stop=True,
                perf_mode=DR,
            )

        ecnt = 0  # evac-engine cursor
        ocnt = 0  # out-ring cursor
        for n in range(NB):
            last = n == NB - 1
            for c in range(NCH):
                for m in range(2):
                    o = opool.tile(
                        [P, CH], OUT_DT, tag=f"o{m}{c}", name=f"o{n}{m}{c}",
                        bufs=2,
                    )
                    # Two 1-bank-deep [128, 1024] PSUM tiles per (c, m);
                    # each takes two single-shot DoubleRow matmuls (full
                    # K=256 contraction per instruction).
                    ps = [
                        pspool.tile(
                            [P, 2 * FD], mybir.dt.float32, tag=f"ps{'AB'[h]}",
                            name="ps",
                        )
                        for h in range(2)
                    ]
                    for h in range(2):
                        for jj in range(2):
                            j = h * 2 + jj
                            nc.tensor.matmul(
                                ps[h][:, jj * FD : (jj + 1) * FD],
                                lhsT=wtile[:, :, m * P : (m + 1) * P],
                                rhs=a[n, c][:, :, j * FD : (j + 1) * FD],
                                start=True,
                                stop=True,
                                perf_mode=DR,
                            )
                    # Evacuate fp32 -> e3m4 with fused *SO/SW; the two
                    # halves go to the next two engines in the weighted
                    # rotation (scalar 17 / vector 15).
                    fine = last and c == NCH - 1
                    for h in range(2):
                        evac(
                            evac_engines[ecnt % len(evac_engines)],
                            o[:, h * 2 * FD : (h + 1) * 2 * FD],
                            ps[h][:],
                            SO / SW,
                        )
                        ecnt += 1
                        if fine:
                            # Final chunk: per-half out-DMAs issued as the
                            # halves finish, so the completion-lagged LAST
                            # transfer is half-size and starts right after
                            # the last evacuation (the very last rides the
                            # idle sync ring).
                            ring = nc.sync if (m == 1 and h == 1) else nc.gpsimd
                            ring.dma_start(
                                out=out[
                                    n, m, :,
                                    c * CH + h * 2 * FD : c * CH + (h + 1) * 2 * FD,
                                ],
                                in_=o[:, h * 2 * FD : (h + 1) * 2 * FD],
                            )
                    if not fine:
                        rings = drain_rings if last else out_rings
                        ring = rings[ocnt % len(rings)]
                        ocnt += 1
                        ring.dma_start(
                            out=out[n, m, :, c * CH : (c + 1) * CH],
                            in_=o[:],
                        )


_NC_CACHE = {}


def _get_nc(cfg=None):
    key = tuple(sorted(dict(CFG, **(cfg or {})).items()))
    if key not in _NC_CACHE:
        nc = bacc.Bacc(
            "TRN2", debug=False, enable_asserts=False, enable_partition_id=False
        )
        act = nc.dram_tensor("act", [NB, 2, P, HW], IN_DT, kind="ExternalInput").ap()
        w = nc.dram_tensor("w", [P, 2, C], IN_DT, kind="ExternalInput").ap()
        out = nc.dram_tensor("out", [NB, 2, P, HW], OUT_DT, kind="ExternalOutput").ap()
        with tile.TileContext(nc) as tc:
            _body(tc, out, act, w, cfg)
        nc.compile()
        _NC_CACHE[key] = nc
    return _NC_CACHE[key]


def _run(activations: np.ndarray, w: np.ndarray, trace: bool = False, cfg=None):
    act32 = np.ascontiguousarray(activations, dtype=np.float32)
    acts8 = act32.reshape(NCORES, NB, 2, P, HW).astype(NP_IN)
    # E = Winv - I, scaled into e4m3 normal range and packed [128, 2, 256]:
    # wp[p, i, m] = E[i*128+p, m] * SW.
    E = (w.astype(np.float64) - np.eye(C)) * SW
    wp = np.ascontiguousarray(
        E.astype(np.float32).reshape(2, P, C).transpose(1, 0, 2).astype(NP_IN)
    )
    in_maps = [{"act": acts8[i], "w": wp} for i in range(NCORES)]
    nc = _get_nc(cfg)
    res = run_bass_kernel_spmd(nc, in_maps, list(range(NCORES)), trace=trace)
    corr = np.stack([res.results[i]["out"] for i in range(NCORES)], axis=0)
    out = act32 + corr.astype(np.float32).reshape(N, C, H, W) * np.float32(1.0 / SO)
    return out, res


def kernel(activations: np.ndarray, inhibition_filter: np.ndarray) -> np.ndarray:
    w = _build_w(inhibition_filter)
    out, _ = _run(activations, w, trace=False)
    return out
